# revision 28
# baseline (speedup 1.0000x reference)
"""IsoMaxPlus first-part kernel for TRN2 (8 NeuronCores, data-parallel on B).

out[b, c] = -|s| * sqrt(max(2 - 2 * <f_b/||f_b||, p_c/||p_c||>, 1e-12))

Host prep (per core shard of 8192 rows): features are cast to fp8-e4m3,
row-interleaved pairwise (partition p of block pair n holds rows 256n+2p
and 256n+2p+1 so a 2-block store writes 4KB contiguous per partition),
and pre-transposed to d-major layout [128 dpart, 64 blocks, 4 kchunk, 128];
prototypes are zero-padded to [1024, 512] and cast to fp8-e4m3. The device
output is fp16 (upcast to fp32 on host); tolerance budget allows it.

Device per core (v4 — fp8 DoubleRow everywhere, 5-engine split):
  prolog: one DMA stages all features (32KB/part); prototypes are row-
          normalized (ACT square+accum heads the queue, negated scale
          folded in) and transposed 128x128-wise on the TensorEngine into
          pnT [128, 4, 1024] fp8 (GpSimd cannot touch PSUM).
  norms:  per 128-row block the PE computes the fp8 DoubleRow gram
          G = fT^T fT (K=256 x2, exact fp8 squares in fp32 psum); DVE
          masks G with the identity to bf16 (TensorTensorReduce faults on
          HW); a PE ones-matmul drops n2 = diag(G) into a psum column.
          Per group of 8 blocks ACT sqrt produces ||f|| and (with DVE
          reciprocal) ta = -|s|*sqrt(2/||f||).
  dots:   per block 4 DoubleRow matmuls (fp8 x fp8, K=256 each, 0.5
          cyc/col) accumulate d = -f.p_hat into psum [128,1000]; ACT
          computes sqrt(d + ||f||) with immediate scale (out = ta *
          sqrt(d+n) = -|s|sqrt(2-2<f^,p^>)) straight to fp16; DVE applies
          ta per block in 4x mode; stores go out 2 blocks per DMA on
          alternating SP-HWDGE / GpSimd-SWDGE queues, 4KB/partition.
Budget/core: ACT 64 sqrt ~ 70us (bottleneck), DMA 21.1MB ~ 70us,
PE ~ 55us, DVE ~ 42us; vs 173us mixed-precision baseline.
"""

import numpy as np
from contextlib import ExitStack

import ml_dtypes

import concourse.bass as bass
import concourse.tile as tile
from concourse import bacc, masks, mybir
from concourse.bass import ts
from concourse.bass_utils import run_bass_kernel_spmd

N_CORES = 8
B, D, C = 65536, 512, 1000
CP = 1024                  # prototypes padded (zeros) for 128-alignment
CPB = CP // 128            # 8 proto chunks
BS = B // N_CORES          # 8192 rows per core
NB = BS // 128             # 64 row blocks
NPAIR = NB // 2            # 32 store pairs
KC = D // 128              # 4 contraction chunks
GB = 8                     # blocks per norm group
NGRP = NB // GB            # 8 groups
NSPLIT = (512, C - 512)    # psum halves (max moving free dim = 512)
F32 = mybir.dt.float32
F16 = mybir.dt.float16
BF16 = mybir.dt.bfloat16
F8 = mybir.dt.float8e4
SQRT = mybir.ActivationFunctionType.Sqrt
SQUARE = mybir.ActivationFunctionType.Square
MULT = mybir.AluOpType.mult
ADD = mybir.AluOpType.add
DR = mybir.MatmulPerfMode.DoubleRow


def _emit(nc):
    f_dram = nc.dram_tensor("features", [128, NB, KC, 128], F8, kind="ExternalInput").ap()
    p_dram = nc.dram_tensor("prototypes", [CP, D], F8, kind="ExternalInput").ap()
    s_dram = nc.dram_tensor("distance_scale", [1], F32, kind="ExternalInput").ap()
    o_dram = nc.dram_tensor("out", [BS, C], F16, kind="ExternalOutput").ap()
    # rows are host-interleaved: b = 256n + 2p + j -> 4KB/partition stores
    o_pair = o_dram.rearrange("(n p j) c -> p n j c", n=NPAIR, j=2, p=128)
    # host stores proto row p*CPB+cb = original proto cb*128+p, so each
    # partition line loads CPB*D = 4KB contiguous (big DMA descriptors)
    p_il = p_dram.rearrange("(p cb) d -> p cb d", p=128, cb=CPB)

    with tile.TileContext(nc) as tc, ExitStack() as ctx:
        singles = ctx.enter_context(tc.tile_pool(name="singles", bufs=1))

        fT = singles.tile([128, NB, KC, 128], F8)  # all features (fp8), 32KB/part
        pnT = singles.tile([128, KC, CP], F8)      # -p_hat transposed
        pfull = singles.tile([128, CPB, D], F8)    # raw prototypes, proto-major
        identity = singles.tile([128, 128], BF16)
        nba = singles.tile([128, NB], F32)         # row norms ||f||
        ta = singles.tile([128, NB], F32)          # -|s|*sqrt(2/||f||)
        n2a = singles.tile([128, NB], F32)         # row norms^2
        s_b = singles.tile([128, 1], F32)
        two_s2 = singles.tile([128, 1], F32)

        nc.gpsimd.dma_start(out=s_b[:], in_=s_dram.to_broadcast([128, 1]))
        masks.make_identity(nc, identity[:])
        warm = singles.tile([128, 1], F32)
        nc.gpsimd.memset(warm[:], 1.0)
        nc.scalar.activation(warm[:], warm[:], SQUARE)  # load Square table
        nc.scalar.sqrt(warm[:], warm[:])                # load Sqrt table
        s2t = singles.tile([128, 1], F32)
        nc.vector.tensor_mul(s2t[:], s_b[:], s_b[:])
        nc.vector.tensor_scalar_mul(two_s2[:], s2t[:], 2.0)

        # Load order on the sync HWDGE queue: protos first (gate the whole
        # pnT chain), then the first feature groups (gate norms/dots).
        nc.sync.dma_start(out=pfull[:], in_=p_il[:])
        nc.sync.dma_start(out=fT[:, ts(0, GB)], in_=f_dram[:, ts(0, GB)])
        nc.sync.dma_start(out=fT[:, ts(1, GB)], in_=f_dram[:, ts(1, GB)])

        gpsum = ctx.enter_context(tc.tile_pool(name="gpsum", bufs=2, space="PSUM"))
        gscr = ctx.enter_context(tc.tile_pool(name="gscr", bufs=3))
        gsml = ctx.enter_context(tc.tile_pool(name="gsml", bufs=2))
        opool = ctx.enter_context(tc.tile_pool(name="opool", bufs=4))

        # ---- prototypes: row norms + fused normalize-transpose ----
        # Norms: even chunks ACT square+accum, odd chunks GpSimd square +
        # DVE reduce (keeps every queue short). Normalize+transpose happen
        # in ONE PE matmul per (cb,kc): stationary = raw proto-major chunk,
        # moving = identity * (-1/||p_c||) built per-partition on DVE; the
        # psum result IS the scaled transposed chunk, cast-copied to pnT.
        pna = singles.tile([128, CPB], F32)
        with tc.tile_pool(name="tpsum", bufs=3, space="PSUM") as tpsum, \
             tc.tile_pool(name="ppool", bufs=4) as ppool, \
             tc.tile_pool(name="psml", bufs=4) as psml:
            # GpSimd squares (odd chunks) all start as soon as data lands;
            # ACT takes the even chunks. Each half of the chain completes
            # fully (squares -> smalls -> scaled transpose -> pnT copies)
            # before ACT moves to the next half, so the lo<512 dots can
            # start on half 0 alone.
            for cb in (1, 3, 5, 7):
                psq = ppool.tile([128, D], BF16, tag="psq", name="psq")
                nc.gpsimd.tensor_tensor(psq[:], pfull[:, cb, :],
                                        pfull[:, cb, :], op=MULT)
                nc.vector.tensor_reduce(pna[:, cb : cb + 1], psq[:],
                                        axis=mybir.AxisListType.X, op=ADD)

            def emit_pchunk(cb, npri):
                rdiag = psml.tile([128, 128], BF16, tag="rdiag")
                nc.vector.tensor_scalar(
                    out=rdiag[:], in0=identity[:],
                    scalar1=npri[:, cb % 4 : cb % 4 + 1],
                    scalar2=None, op0=MULT,
                )
                pst = tpsum.tile([128, D], F32, tag="pst")
                for kc in range(KC):
                    nc.tensor.matmul(
                        pst[:, ts(kc, 128)], pfull[:, cb, ts(kc, 128)],
                        rdiag[:], skip_group_check=True,
                    )
                dst = pnT[:, :, ts(cb, 128)]
                srcv = pst[:].rearrange("p (kc x) -> p kc x", kc=KC)
                if cb % 2 == 0:
                    nc.scalar.copy(dst, srcv)
                else:
                    nc.vector.tensor_copy(dst, srcv)

            for half in range(2):
                for cb in (4 * half, 4 * half + 2):
                    psq = ppool.tile([128, D], BF16, tag="psq", name="psq")
                    nc.scalar.activation(psq[:], pfull[:, cb, :], SQUARE,
                                         accum_out=pna[:, cb : cb + 1])
                pn = psml.tile([128, 4], F32, tag="pn")
                nc.scalar.activation(pn[:], pna[:, 4 * half : 4 * half + 4],
                                     SQRT)
                nc.vector.tensor_scalar_max(pn[:], pn[:], 1e-12)
                npri = psml.tile([128, 4], F32, tag="npri", name="npri")
                nc.vector.reciprocal(npri[:], pn[:])
                nc.vector.tensor_scalar_mul(npri[:], npri[:], -1.0)
                for cb in range(4 * half, 4 * half + 4):
                    emit_pchunk(cb, npri)

        for gi in range(2, NGRP):
            nc.sync.dma_start(out=fT[:, ts(gi, GB)], in_=f_dram[:, ts(gi, GB)])

        sc_tiles = {}

        def emit_norm_front(ib):
            # n2 = diag(fT^T fT): exact fp8 squares via DoubleRow gram on the
            # PE (2 instrs, K=256 each); DVE masks with the identity to a
            # bf16 one-nonzero-per-column matrix.
            G = gpsum.tile([128, 128], F32, tag="gram")
            for kp in range(2):
                nc.tensor.matmul(
                    G[:], fT[:, ib, 2 * kp : 2 * kp + 2, :],
                    fT[:, ib, 2 * kp : 2 * kp + 2, :],
                    start=(kp == 0), stop=(kp == 1),
                    perf_mode=DR, skip_group_check=True,
                )
            sc = gscr.tile([128, 128], F32, tag="gscr")
            nc.vector.tensor_tensor(sc[:], G[:], identity[:], op=MULT)
            sc_tiles[ib] = sc

        def emit_norm_back(ib):
            # DVE row-reduce of the masked gram -> n2 in SBUF (keeps the PE
            # queue free for dots; no psum bank needed)
            nc.vector.tensor_reduce(n2a[:, ib : ib + 1], sc_tiles.pop(ib)[:],
                                    axis=mybir.AxisListType.X, op=ADD)

        def emit_smalls(lo, n):
            # nba = ||f||; ta = -|s|*sqrt(2/||f||)  (out = ta * sqrt(d+nba))
            nc.scalar.activation(nba[:, lo : lo + n], n2a[:, lo : lo + n], SQRT)
            rin = gsml.tile([128, n], F32, tag=f"rin{n}", name="rin")
            nc.vector.reciprocal(rin[:], nba[:, lo : lo + n])
            tg = gsml.tile([128, n], F32, tag=f"tg{n}", name="tg")
            nc.scalar.activation(tg[:], rin[:], SQRT, scale=two_s2[:])
            nc.vector.tensor_scalar_mul(ta[:, lo : lo + n], tg[:], -1.0)

        # prolog norms: only blocks 0-3 (pairs 0-1); the rest ride the pair
        # loop so the early PE/DVE queues stay short.
        for ib in range(4):
            emit_norm_front(ib)
        for ib in range(4):
            emit_norm_back(ib)
        emit_smalls(0, 2)
        emit_smalls(2, 2)

        mpsum = ctx.enter_context(tc.tile_pool(name="mpsum", bufs=3, space="PSUM"))

        def emit_pair(pair):
            e2 = opool.tile([128, 2, C], F16, tag="e2")
            for h in range(2):
                ib = 2 * pair + h
                dots = mpsum.tile([128, C], F32, tag="dots")
                for lo, width in ((0, NSPLIT[0]), (NSPLIT[0], NSPLIT[1])):
                    for kp in range(2):
                        nc.tensor.matmul(
                            dots[:, lo : lo + width],
                            fT[:, ib, 2 * kp : 2 * kp + 2, :],
                            pnT[:, 2 * kp : 2 * kp + 2, lo : lo + width],
                            start=(kp == 0), stop=(kp == 1),
                            perf_mode=DR, skip_group_check=True,
                        )
                nc.scalar.activation(
                    e2[:, h], dots[:], SQRT, bias=nba[:, ib : ib + 1],
                )
                # fold -|s|*sqrt(2/n) in per block (fp16 in/out: DVE 4x mode)
                nc.vector.tensor_scalar_mul(e2[:, h], e2[:, h],
                                            ta[:, ib : ib + 1])
            eng = nc.sync if pair % 2 == 0 else nc.gpsimd
            eng.dma_start(out=o_pair[:, pair], in_=e2[:])

        # Norm lookahead: pairs 0-3 carry two norm block-pairs (+2 fills
        # the gap left by the short prolog, +6 is the steady lookahead);
        # smalls run per-q early (2 cols) and per-group later (8 cols), one
        # pair behind their producers so the ACT queue never head-blocks.
        def norm_qs(pair):
            qs = []
            if pair < 4:
                qs.append(pair + 2)
            if pair + 6 < NPAIR:
                qs.append(pair + 6)
            return qs

        pending = []
        for pair in range(NPAIR):
            for lo, n in pending:
                emit_smalls(lo, n)
            pending = []
            # dots/sqrt/store first so PE never idles behind norm work
            emit_pair(pair)
            for q in norm_qs(pair):
                emit_norm_front(2 * q)
                emit_norm_front(2 * q + 1)
                emit_norm_back(2 * q)
                emit_norm_back(2 * q + 1)
                if q <= 7:
                    pending.append((2 * q, 2))
                elif q % 4 == 3:
                    pending.append((8 * (q // 4), 8))

def build():
    nc = bacc.Bacc("TRN2", target_bir_lowering=False, debug=False,
                   num_devices=N_CORES)
    _emit(nc)
    nc.compile()
    return nc


def _ensure_ntff_hook():
    """Dev-only: restore the axon NTFF profile hook that the trimmed agent
    image's antenv package lacks, so trace=True yields real HW timings."""
    import sys
    import types

    try:
        from antenv.axon_hooks import get_axon_ntff_profile_hook  # noqa: F401
        return
    except ImportError:
        pass
    from trn_agent_boot.trn_boot import _ntff_profile_via_ctypes

    hook = _ntff_profile_via_ctypes("/opt/axon/libaxon_pjrt.so")
    mod = types.ModuleType("antenv.axon_hooks")
    mod.get_axon_ntff_profile_hook = lambda: hook
    mod.set_axon_ntff_profile_hook = lambda h: None
    sys.modules["antenv.axon_hooks"] = mod


def _prep_features(shard):
    x = shard.astype(ml_dtypes.float8_e4m3)  # saturating cast, matches TRN fp8e4
    # pairwise row interleave: block 2n+j, partition p <- row 256n + 2p + j
    x = x.reshape(NPAIR, 128, 2, D).transpose(0, 2, 1, 3).reshape(NB, 128, D)
    return np.ascontiguousarray(x.reshape(NB, 128, KC, 128).transpose(3, 0, 2, 1))


def run(inputs, trace=False):
    if trace:
        _ensure_ntff_hook()
    feats = np.asarray(inputs["features"], dtype=np.float32)
    protos = np.asarray(inputs["prototypes"], dtype=np.float32)
    dscale = np.ascontiguousarray(np.asarray(inputs["distance_scale"], dtype=np.float32))
    protos_p = np.zeros((CP, D), dtype=ml_dtypes.float8_e4m3)
    protos_p[:C] = protos.astype(ml_dtypes.float8_e4m3)
    # interleave so dram row p*CPB+cb = proto cb*128+p (4KB per partition)
    protos_p = np.ascontiguousarray(
        protos_p.reshape(CPB, 128, D).transpose(1, 0, 2).reshape(CP, D))
    nc = build()
    in_maps = [
        {
            "features": _prep_features(feats[i * BS : (i + 1) * BS]),
            "prototypes": protos_p,
            "distance_scale": dscale,
        }
        for i in range(N_CORES)
    ]
    res = run_bass_kernel_spmd(nc, in_maps, core_ids=list(range(N_CORES)),
                               trace=trace)
    out = np.concatenate([r["out"] for r in res.results], axis=0).astype(np.float32)
    return out, res


def kernel(**inputs) -> np.ndarray:
    out, _ = run(inputs, trace=False)
    return out
